# revision 18
# baseline (speedup 1.0000x reference)
"""Trainium2 Bass kernel for nn_BatchDistance (pairwise joint-entropy matrix).

Math: for x strictly positive, with L = x * log(x) (elementwise over [n, d]):
    ent(i, j) = -sum_d x[i,d]*x[j,d]*(log x[i,d] + log x[j,d])
              = -(L[i] . x[j] + x[i] . L[j])
Stack per-point feature vectors g_p = [x_p ; L_p] (len 2d=128) and
h_p = -[L_p ; x_p]; then ent(i,j) = h_i . g_j  -- a single K=128 fp32 matmul
per output tile (the K=128 contraction uses the full PE partition dim).

Sharding: each of the 8 cores owns a 256-row block of the symmetric output
and computes the wrapped band D[i, i..i+1024 (mod n)]; the host mirrors the
band into the full matrix (D + D.T coverage, D symmetric).
"""

import numpy as np

from concourse import bass, bacc, mybir, tile
from concourse.bass_utils import run_bass_kernel_spmd

N = 2048
D = 64
NCORES = 8
S = N // NCORES          # 256 rows per core
TPC = S // 128           # row tiles (of 128) per core
BAND = N // 2            # 1024: band half-width, covers all pairs via symmetry
OW = 128 + BAND          # 1152: output width per row-tile
WIN = S + BAND           # 1280: input window per core
F32 = mybir.dt.float32
BF16 = mybir.dt.bfloat16
MMW = 512                # max matmul output chunk width (one fp32 PSUM bank)
CHUNKS = [(0, 288), (288, 288), (576, 288), (864, 288)]  # (off, w) covering OW=1152
N_WARM = 5               # dummy bf16 matmuls to lift the PE HAM clock gate
NGC = 4                  # gw DMA/ln/mul chunking
GC = WIN // NGC          # 320

_compiled = {}


def _build_nc():
    nc = bacc.Bacc("TRN2", target_bir_lowering=False, debug=False)

    xw_in = nc.dram_tensor("xw_in", [64, WIN], F32, kind="ExternalInput").ap()
    out = nc.dram_tensor("out", [TPC, 128, OW], F32, kind="ExternalOutput").ap()

    chunks = CHUNKS

    with tile.TileContext(nc) as tc:
        with (
            tc.tile_pool(name="sbuf", bufs=1) as pool,
            tc.tile_pool(name="psum", bufs=min(7, 2 * len(chunks)), space="PSUM") as psum,
            tc.tile_pool(name="wpsum", bufs=1, space="PSUM") as wpsum,
        ):
            gw = pool.tile([128, WIN], F32)
            hr = pool.tile([128, S], F32)
            tln = pool.tile([64, WIN], F32)

            # PE warm-up: HAM keeps the PE clock-gated at 1.2 GHz until it has
            # been busy ~3.4us; dummy bf16 matmuls on a zero tile lift the gate
            # while the input DMA + ln/mul prologue runs, so the real fp32
            # matmuls stream at 2.4 GHz.
            wz = pool.tile([128, MMW], BF16)
            nc.vector.memset(wz[:], 0.0)
            wps = wpsum.tile([128, MMW], F32)
            for _ in range(N_WARM):
                nc.tensor.matmul(wps[:], wz[:, 0:128], wz[:], start=True, stop=True)

            # gw := [x ; L], loaded/processed in chunks along the window
            for k in range(NGC):
                cs = bass.ts(k, GC)
                nc.sync.dma_start(gw[0:64, cs], xw_in[:, cs])
            # hr := -[L ; x] for the core's own rows = window cols [0:S)
            nc.vector.tensor_scalar_mul(hr[64:128, :], gw[0:64, 0:S], -1.0)
            for k in range(NGC):
                cs = bass.ts(k, GC)
                nc.scalar.activation(
                    tln[:, cs], gw[0:64, cs], mybir.ActivationFunctionType.Ln
                )
                nc.vector.tensor_mul(gw[64:128, cs], gw[0:64, cs], tln[:, cs])
                if k == 0:
                    # hr lower half = -(x * ln x) off the first chunk's ln
                    nc.vector.scalar_tensor_tensor(
                        hr[0:64, :], tln[:, 0:S], -1.0, gw[0:64, 0:S],
                        mybir.AluOpType.mult, mybir.AluOpType.mult,
                    )

            for ci, (off, w) in enumerate(chunks):
                oc = pool.tile([128, TPC, w], F32, tag=f"oc{ci}", bufs=1)
                for t in range(TPC):
                    ps = psum.tile([128, MMW], F32, tag="ps")
                    nc.tensor.matmul(
                        ps[:, 0:w],
                        hr[:, t * 128 : (t + 1) * 128],
                        gw[:, t * 128 + off : t * 128 + off + w],
                        start=True,
                        stop=True,
                    )
                    # copy PSUM -> SBUF (DMA cannot read PSUM)
                    if (ci + t) % 2 == 0:
                        nc.vector.tensor_copy(oc[:, t, :], ps[:, 0:w])
                    else:
                        nc.scalar.copy(oc[:, t, :], ps[:, 0:w])
                # one DMA stores this chunk for both row tiles:
                # SBUF [128, 2, w] -> DRAM [2, 128, w]
                deng = nc.sync if ci % 2 == 0 else nc.scalar
                deng.dma_start(
                    out[:, :, off : off + w].rearrange("t p c -> p t c"),
                    oc[:],
                )

    nc.compile()
    return nc


def _prep_inputs(x1):
    """Per-core input maps. x1: [N, D] float32."""
    xT = np.ascontiguousarray(x1.T)  # [64, N]
    in_maps = []
    for c in range(NCORES):
        s = S * c
        wcols = (s + np.arange(WIN)) % N
        in_maps.append({"xw_in": np.ascontiguousarray(xT[:, wcols])})
    return in_maps


def _assemble(results, dtype):
    """Scatter per-core band outputs into the full symmetric matrix."""
    full = np.empty((N, N), dtype=dtype)
    blocks = []
    for c in range(NCORES):
        o = results[c]["out"]  # [TPC, 128, OW]
        for t in range(TPC):
            blocks.append((S * c + 128 * t, o[t]))
    # Direct writes: D[s:s+128, s:s+OW (mod N)] = block
    for s, blk in blocks:
        e = s + OW
        if e <= N:
            full[s : s + 128, s:e] = blk
        else:
            full[s : s + 128, s:N] = blk[:, : N - s]
            full[s : s + 128, 0 : e - N] = blk[:, N - s :]
    # Mirror writes: D[s:s+OW (mod N), s:s+128] = block.T
    for s, blk in blocks:
        bt = blk.T
        e = s + OW
        if e <= N:
            full[s:e, s : s + 128] = bt
        else:
            full[s:N, s : s + 128] = bt[: N - s, :]
            full[0 : e - N, s : s + 128] = bt[N - s :, :]
    return full


def _run(x1):
    x1 = np.ascontiguousarray(np.asarray(x1, dtype=np.float32))
    assert x1.shape == (N, D)
    if "nc" not in _compiled:
        _compiled["nc"] = _build_nc()
    nc = _compiled["nc"]
    in_maps = _prep_inputs(x1)
    res = run_bass_kernel_spmd(nc, in_maps, list(range(NCORES)))
    full = _assemble(res.results, x1.dtype)
    return full, res


def kernel(x1):
    full, _ = _run(x1)
    return full


# revision 19
# speedup vs baseline: 1.0126x; 1.0126x over previous
"""Trainium2 Bass kernel for nn_BatchDistance (pairwise joint-entropy matrix).

Math: for x strictly positive, with L = x * log(x) (elementwise over [n, d]):
    ent(i, j) = -sum_d x[i,d]*x[j,d]*(log x[i,d] + log x[j,d])
              = -(L[i] . x[j] + x[i] . L[j])
Stack per-point feature vectors g_p = [x_p ; L_p] (len 2d=128) and
h_p = -[L_p ; x_p]; then ent(i,j) = h_i . g_j  -- a single K=128 fp32 matmul
per output tile (the K=128 contraction uses the full PE partition dim).

Sharding: each of the 8 cores owns a 256-row block of the symmetric output
and computes the wrapped band D[i, i..i+1024 (mod n)]; the host mirrors the
band into the full matrix (D + D.T coverage, D symmetric).
"""

import numpy as np

from concourse import bass, bacc, mybir, tile
from concourse.bass_utils import run_bass_kernel_spmd

N = 2048
D = 64
NCORES = 8
S = N // NCORES          # 256 rows per core
TPC = S // 128           # row tiles (of 128) per core
BAND = N // 2            # 1024: band half-width, covers all pairs via symmetry
OW = 128 + BAND          # 1152: output width per row-tile
WIN = S + BAND           # 1280: input window per core
F32 = mybir.dt.float32
BF16 = mybir.dt.bfloat16
MMW = 512                # max matmul output chunk width (one fp32 PSUM bank)
CHUNKS = [(0, 288), (288, 288), (576, 288), (864, 288)]  # (off, w) covering OW=1152
N_WARM = 5               # dummy bf16 matmuls to lift the PE HAM clock gate
NGC = 4                  # gw DMA/ln/mul chunking
GC = WIN // NGC          # 320

_compiled = {}


def _build_nc():
    nc = bacc.Bacc("TRN2", target_bir_lowering=False, debug=False)

    xw_in = nc.dram_tensor("xw_in", [64, WIN], F32, kind="ExternalInput").ap()
    out = nc.dram_tensor("out", [TPC, 128, OW], F32, kind="ExternalOutput").ap()

    chunks = CHUNKS

    with tile.TileContext(nc) as tc:
        with (
            tc.tile_pool(name="sbuf", bufs=1) as pool,
            tc.tile_pool(name="psum", bufs=min(7, 2 * len(chunks)), space="PSUM") as psum,
            tc.tile_pool(name="wpsum", bufs=1, space="PSUM") as wpsum,
        ):
            gw = pool.tile([128, WIN], F32)
            hr = pool.tile([128, S], F32)
            tln = pool.tile([64, WIN], F32)

            # PE warm-up: HAM keeps the PE clock-gated at 1.2 GHz until it has
            # been busy ~3.4us; dummy bf16 matmuls on a zero tile lift the gate
            # while the input DMA + ln/mul prologue runs, so the real fp32
            # matmuls stream at 2.4 GHz.
            wz = pool.tile([128, MMW], BF16)
            nc.vector.memset(wz[:], 0.0)
            wps = wpsum.tile([128, MMW], F32)
            for _ in range(N_WARM):
                nc.tensor.matmul(wps[:], wz[:, 0:128], wz[:], start=True, stop=True)

            # gw := [x ; L], loaded/processed in chunks along the window
            for k in range(NGC):
                cs = bass.ts(k, GC)
                nc.sync.dma_start(gw[0:64, cs], xw_in[:, cs])
            # hr := -[L ; x] for the core's own rows = window cols [0:S)
            nc.vector.tensor_scalar_mul(hr[64:128, :], gw[0:64, 0:S], -1.0)

            def emit_chunk(ci, off, w):
                # matmuls + PSUM->SBUF copies + one merged store for chunk ci
                oc = pool.tile(
                    [128, TPC, w], F32, tag=f"oc{ci}", bufs=1, name=f"oc{ci}"
                )
                for t in range(TPC):
                    ps = psum.tile([128, MMW], F32, tag="ps", name="ps")
                    nc.tensor.matmul(
                        ps[:, 0:w],
                        hr[:, t * 128 : (t + 1) * 128],
                        gw[:, t * 128 + off : t * 128 + off + w],
                        start=True,
                        stop=True,
                    )
                    if (ci + t) % 2 == 0:
                        nc.vector.tensor_copy(oc[:, t, :], ps[:, 0:w])
                    else:
                        nc.scalar.copy(oc[:, t, :], ps[:, 0:w])
                # SBUF [128, 2, w] -> DRAM [2, 128, w]
                deng = nc.sync if ci % 2 == 0 else nc.scalar
                deng.dma_start(
                    out[:, :, off : off + w].rearrange("t p c -> p t c"),
                    oc[:],
                )

            # Interleave window processing with per-chunk matmul/store emission
            # so the scheduler overlaps the out-corridor with the prologue.
            # Chunk ci's matmuls need gw cols [off, off+128*(TPC-1)+w) ready.
            emitted = 0
            for k in range(NGC):
                cs = bass.ts(k, GC)
                nc.scalar.activation(
                    tln[:, cs], gw[0:64, cs], mybir.ActivationFunctionType.Ln
                )
                nc.vector.tensor_mul(gw[64:128, cs], gw[0:64, cs], tln[:, cs])
                if k == 0:
                    # hr lower half = -(x * ln x) off the first chunk's ln
                    nc.vector.scalar_tensor_tensor(
                        hr[0:64, :], tln[:, 0:S], -1.0, gw[0:64, 0:S],
                        mybir.AluOpType.mult, mybir.AluOpType.mult,
                    )
                ready = (k + 1) * GC
                while emitted < len(chunks):
                    off, w = chunks[emitted]
                    if off + 128 * (TPC - 1) + w > ready:
                        break
                    emit_chunk(emitted, off, w)
                    emitted += 1
            while emitted < len(chunks):
                off, w = chunks[emitted]
                emit_chunk(emitted, off, w)
                emitted += 1

    nc.compile()
    return nc


def _prep_inputs(x1):
    """Per-core input maps. x1: [N, D] float32."""
    xT = np.ascontiguousarray(x1.T)  # [64, N]
    in_maps = []
    for c in range(NCORES):
        s = S * c
        wcols = (s + np.arange(WIN)) % N
        in_maps.append({"xw_in": np.ascontiguousarray(xT[:, wcols])})
    return in_maps


def _assemble(results, dtype):
    """Scatter per-core band outputs into the full symmetric matrix."""
    full = np.empty((N, N), dtype=dtype)
    blocks = []
    for c in range(NCORES):
        o = results[c]["out"]  # [TPC, 128, OW]
        for t in range(TPC):
            blocks.append((S * c + 128 * t, o[t]))
    # Direct writes: D[s:s+128, s:s+OW (mod N)] = block
    for s, blk in blocks:
        e = s + OW
        if e <= N:
            full[s : s + 128, s:e] = blk
        else:
            full[s : s + 128, s:N] = blk[:, : N - s]
            full[s : s + 128, 0 : e - N] = blk[:, N - s :]
    # Mirror writes: D[s:s+OW (mod N), s:s+128] = block.T
    for s, blk in blocks:
        bt = blk.T
        e = s + OW
        if e <= N:
            full[s:e, s : s + 128] = bt
        else:
            full[s:N, s : s + 128] = bt[: N - s, :]
            full[0 : e - N, s : s + 128] = bt[N - s :, :]
    return full


def _run(x1):
    x1 = np.ascontiguousarray(np.asarray(x1, dtype=np.float32))
    assert x1.shape == (N, D)
    if "nc" not in _compiled:
        _compiled["nc"] = _build_nc()
    nc = _compiled["nc"]
    in_maps = _prep_inputs(x1)
    res = run_bass_kernel_spmd(nc, in_maps, list(range(NCORES)))
    full = _assemble(res.results, x1.dtype)
    return full, res


def kernel(x1):
    full, _ = _run(x1)
    return full


# revision 22
# speedup vs baseline: 1.0314x; 1.0186x over previous
"""Trainium2 Bass kernel for nn_BatchDistance (pairwise joint-entropy matrix).

Math: for x strictly positive, with L = x * log(x) (elementwise over [n, d]):
    ent(i, j) = -sum_d x[i,d]*x[j,d]*(log x[i,d] + log x[j,d])
              = -(L[i] . x[j] + x[i] . L[j])
Stack per-point feature vectors g_p = [x_p ; L_p] (len 2d=128) and
h_p = -[L_p ; x_p]; then ent(i,j) = h_i . g_j  -- a single K=128 fp32 matmul
per output tile (the K=128 contraction uses the full PE partition dim).

Sharding: each of the 8 cores owns a 256-row block of the symmetric output
and computes the wrapped band D[i, i..i+1024 (mod n)]; the host mirrors the
band into the full matrix (D + D.T coverage, D symmetric).
"""

import numpy as np

from concourse import bass, bacc, mybir, tile
from concourse.bass_utils import run_bass_kernel_spmd

N = 2048
D = 64
NCORES = 8
S = N // NCORES          # 256 rows per core
TPC = S // 128           # row tiles (of 128) per core
BAND = N // 2            # 1024: band half-width, covers all pairs via symmetry
OW = 128 + BAND          # 1152: output width per row-tile
WIN = S + BAND           # 1280: input window per core
F32 = mybir.dt.float32
BF16 = mybir.dt.bfloat16
MMW = 512                # max matmul output chunk width (one fp32 PSUM bank)
CHUNKS = [(0, 288), (288, 288), (576, 288), (864, 288)]  # (off, w) covering OW=1152
N_WARM = 5               # dummy bf16 matmuls to lift the PE HAM clock gate
NGC = 4                  # gw DMA/ln/mul chunking
GC = WIN // NGC          # 320

_compiled = {}


def _build_nc():
    nc = bacc.Bacc("TRN2", target_bir_lowering=False, debug=False)

    xw_in = nc.dram_tensor("xw_in", [64, WIN], F32, kind="ExternalInput").ap()
    out = nc.dram_tensor("out", [TPC, 128, OW], F32, kind="ExternalOutput").ap()

    chunks = CHUNKS

    with tile.TileContext(nc) as tc:
        with (
            tc.tile_pool(name="sbuf", bufs=1) as pool,
            tc.tile_pool(name="psum", bufs=min(7, 2 * len(chunks)), space="PSUM") as psum,
            tc.tile_pool(name="wpsum", bufs=1, space="PSUM") as wpsum,
        ):
            gw = pool.tile([128, WIN], F32)
            hr = pool.tile([128, S], F32)
            tln = pool.tile([64, WIN], F32)

            # PE warm-up: HAM keeps the PE clock-gated at 1.2 GHz until it has
            # been busy ~3.4us; dummy bf16 matmuls on a zero tile lift the gate
            # while the input DMA + ln/mul prologue runs, so the real fp32
            # matmuls stream at 2.4 GHz.
            wz = pool.tile([128, MMW], BF16)
            nc.vector.memset(wz[:], 0.0)
            wps = wpsum.tile([128, MMW], F32)
            for _ in range(N_WARM):
                nc.tensor.matmul(wps[:], wz[:, 0:128], wz[:], start=True, stop=True)

            # gw := [x ; L], loaded/processed in chunks along the window
            for k in range(NGC):
                cs = bass.ts(k, GC)
                nc.sync.dma_start(gw[0:64, cs], xw_in[:, cs])
            # hr := -[L ; x] for the core's own rows = window cols [0:S)
            nc.vector.tensor_scalar_mul(hr[64:128, :], gw[0:64, 0:S], -1.0)

            def emit_chunk(ci, off, w):
                # matmuls + PSUM->SBUF copies + one merged store for chunk ci
                oc = pool.tile(
                    [128, TPC, w], F32, tag=f"oc{ci}", bufs=1, name=f"oc{ci}"
                )
                for t in range(TPC):
                    ps = psum.tile([128, MMW], F32, tag="ps", name="ps")
                    nc.tensor.matmul(
                        ps[:, 0:w],
                        hr[:, t * 128 : (t + 1) * 128],
                        gw[:, t * 128 + off : t * 128 + off + w],
                        start=True,
                        stop=True,
                    )
                    if (ci + t) % 2 == 0:
                        nc.vector.tensor_copy(oc[:, t, :], ps[:, 0:w])
                    else:
                        nc.scalar.copy(oc[:, t, :], ps[:, 0:w])
                # SBUF [128, 2, w] -> DRAM [2, 128, w]
                deng = nc.sync
                deng.dma_start(
                    out[:, :, off : off + w].rearrange("t p c -> p t c"),
                    oc[:],
                )

            # Interleave window processing with per-chunk matmul/store emission
            # so the scheduler overlaps the out-corridor with the prologue.
            # Chunk ci's matmuls need gw cols [off, off+128*(TPC-1)+w) ready.
            emitted = 0
            for k in range(NGC):
                cs = bass.ts(k, GC)
                nc.scalar.activation(
                    tln[:, cs], gw[0:64, cs], mybir.ActivationFunctionType.Ln
                )
                nc.vector.tensor_mul(gw[64:128, cs], gw[0:64, cs], tln[:, cs])
                if k == 0:
                    # hr lower half = -(x * ln x) off the first chunk's ln
                    nc.vector.scalar_tensor_tensor(
                        hr[0:64, :], tln[:, 0:S], -1.0, gw[0:64, 0:S],
                        mybir.AluOpType.mult, mybir.AluOpType.mult,
                    )
                ready = (k + 1) * GC
                while emitted < len(chunks):
                    off, w = chunks[emitted]
                    if off + 128 * (TPC - 1) + w > ready:
                        break
                    emit_chunk(emitted, off, w)
                    emitted += 1
            while emitted < len(chunks):
                off, w = chunks[emitted]
                emit_chunk(emitted, off, w)
                emitted += 1

    nc.compile()
    return nc


def _prep_inputs(x1):
    """Per-core input maps. x1: [N, D] float32."""
    xT = np.ascontiguousarray(x1.T)  # [64, N]
    in_maps = []
    for c in range(NCORES):
        s = S * c
        wcols = (s + np.arange(WIN)) % N
        in_maps.append({"xw_in": np.ascontiguousarray(xT[:, wcols])})
    return in_maps


def _assemble(results, dtype):
    """Scatter per-core band outputs into the full symmetric matrix."""
    full = np.empty((N, N), dtype=dtype)
    blocks = []
    for c in range(NCORES):
        o = results[c]["out"]  # [TPC, 128, OW]
        for t in range(TPC):
            blocks.append((S * c + 128 * t, o[t]))
    # Direct writes: D[s:s+128, s:s+OW (mod N)] = block
    for s, blk in blocks:
        e = s + OW
        if e <= N:
            full[s : s + 128, s:e] = blk
        else:
            full[s : s + 128, s:N] = blk[:, : N - s]
            full[s : s + 128, 0 : e - N] = blk[:, N - s :]
    # Mirror writes: D[s:s+OW (mod N), s:s+128] = block.T
    for s, blk in blocks:
        bt = blk.T
        e = s + OW
        if e <= N:
            full[s:e, s : s + 128] = bt
        else:
            full[s:N, s : s + 128] = bt[: N - s, :]
            full[0 : e - N, s : s + 128] = bt[N - s :, :]
    return full


def _run(x1):
    x1 = np.ascontiguousarray(np.asarray(x1, dtype=np.float32))
    assert x1.shape == (N, D)
    if "nc" not in _compiled:
        _compiled["nc"] = _build_nc()
    nc = _compiled["nc"]
    in_maps = _prep_inputs(x1)
    res = run_bass_kernel_spmd(nc, in_maps, list(range(NCORES)))
    full = _assemble(res.results, x1.dtype)
    return full, res


def kernel(x1):
    full, _ = _run(x1)
    return full
